# revision 18
# baseline (speedup 1.0000x reference)
"""Graphormer attention head on 8 trn2 NeuronCores (row-parallel).

out = softmax(mask(q@k.T/8, adj)) @ v  with q/k/v = x@W+b, adj scattered
from edge_index.

Sharding: core c owns output rows [c*1024, (c+1)*1024). k/v are computed
replicated on every core. All dense math runs in fp16 (PE: 1 cycle/row
vs 4 for fp32): the host ships x^T, the weights and the {0,1} adjacency
mask as fp16, so projections, scores and the attention@V matmuls are all
single-pass fp16 (tolerance is 2e-2; fp16 end-to-end sims at ~9e-4).
The mask is applied multiplicatively AFTER exp on the DVE —
w = (e * exp(-2)) * m — which also folds in a -2 score shift so the
fp16 row-sum accumulator cannot overflow (shift cancels in softmax).
The softmax denominator comes free via a ones-column appended to V.
"""
import os
import sys

for _p in ("/opt/trn_rl_repo", "/root/.axon_site/_ro/trn_rl_repo"):
    if os.path.isdir(_p) and _p not in sys.path:
        sys.path.insert(0, _p)

import numpy as np

import concourse.bass as bass
import concourse.bacc as bacc
import concourse.mybir as mybir
import concourse.tile as tile
from concourse.bass_utils import run_bass_kernel_spmd

N = 8192
DIN = 256
DQ = 64
NCORES = 8
NLOC = N // NCORES          # 1024 rows per core
JT = N // 128               # 64 column tiles of 128
SEG = 512                   # moving-operand max
F32 = mybir.dt.float32
F16 = mybir.dt.float16


def _emit(nc, tc, ctx):
    from concourse.mybir import AluOpType as AO, ActivationFunctionType as AF

    xt = nc.dram_tensor("xt", [DIN, N], F16, kind="ExternalInput")
    xtq = nc.dram_tensor("xtq", [DIN, NLOC], F16, kind="ExternalInput")
    wq = nc.dram_tensor("wq", [DIN, DQ], F16, kind="ExternalInput")
    wk = nc.dram_tensor("wk", [DIN, DQ], F16, kind="ExternalInput")
    wv = nc.dram_tensor("wv", [DIN, DQ], F16, kind="ExternalInput")
    bq = nc.dram_tensor("bq", [DQ, 1], F32, kind="ExternalInput")
    bk = nc.dram_tensor("bk", [DQ, 1], F32, kind="ExternalInput")
    i65 = nc.dram_tensor("i65", [DQ + 1, DQ + 1], F16, kind="ExternalInput")
    maskt = nc.dram_tensor("maskt", [N, NLOC], F16, kind="ExternalInput")
    out = nc.dram_tensor("out", [NLOC, DQ], F32, kind="ExternalOutput")

    pers = ctx.enter_context(tc.tile_pool(name="pers", bufs=1))
    pm = ctx.enter_context(tc.tile_pool(name="pm", bufs=6))
    pe_ = ctx.enter_context(tc.tile_pool(name="pe", bufs=3))
    pw = ctx.enter_context(tc.tile_pool(name="pw", bufs=3))
    pfin = ctx.enter_context(tc.tile_pool(name="pfin", bufs=2))
    ps = ctx.enter_context(tc.tile_pool(name="ps", bufs=2, space="PSUM"))
    pacc = ctx.enter_context(tc.tile_pool(name="pacc", bufs=1, space="PSUM"))
    pp = ctx.enter_context(tc.tile_pool(name="pp", bufs=2, space="PSUM"))

    # ---- persistent SBUF ----
    xt_sb = [pers.tile([128, N], F16, tag=f"xt{c}", name=f"xt{c}") for c in range(2)]
    xtq_sb = [pers.tile([128, NLOC], F16, tag=f"xtq{c}", name=f"xtq{c}") for c in range(2)]
    # small tensors issue from the otherwise-idle ACT/DVE sequencers so the
    # SP stream is free for the big x^T transfers
    w_sb = {}
    for nm, t in (("wq", wq), ("wk", wk), ("wv", wv)):
        for c in range(2):
            w_sb[nm, c] = pers.tile([128, DQ], F16, tag=f"{nm}{c}", name=f"w{nm}{c}")
            nc.scalar.dma_start(w_sb[nm, c][:], t[c * 128:(c + 1) * 128, :])
    bq_sb = pers.tile([DQ, 1], F32, tag="bq")
    bk_sb = pers.tile([DQ, 1], F32, tag="bk")
    i65_sb = pers.tile([DQ + 1, DQ + 1], F16, tag="i65")
    nc.scalar.dma_start(bq_sb[:], bq[:])
    nc.scalar.dma_start(bk_sb[:], bk[:])
    nc.scalar.dma_start(i65_sb[:], i65[:])
    nbias_sb = pers.tile([128, 1], F32, tag="nbias")
    nc.vector.memset(nbias_sb[:], -2.0)
    qt_sb = pers.tile([DQ, NLOC], F16, tag="qt")
    kt_sb = pers.tile([DQ, N], F16, tag="kt")
    vh_sb = pers.tile([128, JT * (DQ + 1)], F16, tag="vh")
    accT_sb = pers.tile([DQ + 1, NLOC], F16, tag="accT")

    # x^T_q and the first x^T segment lead (they unblock q/k projections);
    # 2048-col segments keep the SP issue count low (each dma_start costs
    # ~700ns of SP sequencer time, which gated the old 512-col prologue)
    XSEG = 2048
    for c in range(2):
        nc.sync.dma_start(
            xt_sb[c][:, 0:XSEG], xt[c * 128:(c + 1) * 128, 0:XSEG])
    for c in range(2):
        nc.sync.dma_start(xtq_sb[c][:], xtq[c * 128:(c + 1) * 128, :])
    for s in range(1, N // XSEG):
        for c in range(2):
            nc.sync.dma_start(
                xt_sb[c][:, s * XSEG:(s + 1) * XSEG],
                xt[c * 128:(c + 1) * 128, s * XSEG:(s + 1) * XSEG],
            )

    # ---- projections (all fp16, psum -> fp16 sbuf with bias on DVE) ----
    def _proj(w_name, xs, ncols, dst, bias):
        for s in range(ncols // SEG):
            t = pp.tile([128, SEG], F32, tag="pp", name=f"pp_{w_name}{s}")
            tp = t[:DQ, :]
            nc.tensor.matmul(tp, w_sb[w_name, 0][:], xs[0][:, s * SEG:(s + 1) * SEG],
                             start=True, stop=False)
            nc.tensor.matmul(tp, w_sb[w_name, 1][:], xs[1][:, s * SEG:(s + 1) * SEG],
                             start=False, stop=True)
            nc.vector.tensor_scalar_add(dst[:, s * SEG:(s + 1) * SEG], tp, bias)

    _proj("wq", xtq_sb, NLOC, qt_sb, bq_sb[:])

    def _k_seg(s):
        _proj_seg = slice(s * SEG, (s + 1) * SEG)
        t = pp.tile([128, SEG], F32, tag="pp", name=f"pp_wk{s}")
        tp = t[:DQ, :]
        nc.tensor.matmul(tp, w_sb["wk", 0][:], xt_sb[0][:, _proj_seg],
                         start=True, stop=False)
        nc.tensor.matmul(tp, w_sb["wk", 1][:], xt_sb[1][:, _proj_seg],
                         start=False, stop=True)
        nc.vector.tensor_scalar_add(kt_sb[:, _proj_seg], tp, bk_sb[:])

    _k_seg(0)

    # V [8192 x 64] stored j-major as 64 blocks of [128 x 65] (65th col =
    # 1.0 for the softmax denominator; bv folded in at the end via i65).
    # Groups 2..7 are emitted inside the main loop to shorten the prologue.
    vh3 = vh_sb[:].rearrange("p (b e) -> p b e", e=DQ + 1)
    nc.vector.memset(vh3[:, :, DQ:DQ + 1], 1.0)

    def _v_group(g):
        t = pp.tile([128, SEG], F32, tag="pp", name=f"pp_v{g}")
        for b in range(8):
            jt = g * 8 + b
            o = t[:, b * DQ:(b + 1) * DQ]
            nc.tensor.matmul(o, xt_sb[0][:, jt * 128:(jt + 1) * 128],
                             w_sb["wv", 0][:], start=True, stop=False)
            nc.tensor.matmul(o, xt_sb[1][:, jt * 128:(jt + 1) * 128],
                             w_sb["wv", 1][:], start=False, stop=True)
        gh = vh3[:, g * 8:(g + 1) * 8, 0:DQ]
        nc.vector.tensor_copy(gh, t[:])

    # ---- main loop over 64 column tiles ----
    # PE stream is software-pipelined: scores for jt are emitted before the
    # attention@V matmuls for jt-1, so the PE never sits behind the
    # exp->mask chain of the tile it just scored.
    acc = pacc.tile([DQ + 1, NLOC], F32, tag="acc")

    def _av(jt, w_t):
        vhb = vh3[:, jt, :]
        for h in range(2):
            hs = slice(h * SEG, (h + 1) * SEG)
            nc.tensor.matmul(acc[:, hs], vhb, w_t[:, hs],
                             start=(jt == 0), stop=(jt == JT - 1))

    prev = None
    for jt in range(JT):
        m_t = pm.tile([128, NLOC], F16, tag="m")
        nc.gpsimd.dma_start(m_t[:], maskt[jt * 128:(jt + 1) * 128, :])
        s_t = ps.tile([128, NLOC], F32, tag="s")
        kh = kt_sb[:, jt * 128:(jt + 1) * 128]
        for h in range(2):
            hs = slice(h * SEG, (h + 1) * SEG)
            nc.tensor.matmul(s_t[:, hs], kh, qt_sb[:, hs],
                             start=True, stop=True)
        # remaining K segments / V groups are drip-fed into the PE stream
        # right after the first scores, a safe distance ahead of their
        # consumers, so the cold-start prologue stays minimal
        if jt == 0:
            _k_seg(1)
            _v_group(0)
        elif jt == 1:
            _v_group(1)
        elif jt % 4 == 2 and jt <= 54:
            _k_seg(2 + jt // 4)
        elif jt % 10 == 3 and jt <= 53:
            _v_group(2 + jt // 10)
        if prev is not None:
            _av(*prev)
        e_t = pe_.tile([128, NLOC], F16, tag="e")
        nc.scalar.activation(e_t[:], s_t[:], AF.Exp, bias=nbias_sb[:])
        w_t = pw.tile([128, NLOC], F16, tag="w")
        nc.vector.tensor_tensor(w_t[:], e_t[:], m_t[:], AO.mult)
        prev = (jt, w_t)
    _av(*prev)

    # ---- finish: transpose via matmul with I65 (adds bv*Z), divide by Z ----
    # accT copied in halves and po tiles drawn from both PSUM pools so the
    # 8 transpose->reciprocal->scale->store chains pipeline instead of
    # serializing on a single pool
    nc.scalar.activation(accT_sb[:, 0:SEG], acc[:, 0:SEG], AF.Copy)
    nc.scalar.activation(accT_sb[:, SEG:NLOC], acc[:, SEG:NLOC], AF.Copy)
    for it in range(NLOC // 128):
        pool = pp if it % 2 == 0 else ps
        po = pool.tile([128, DQ + 1], F32, tag="pp" if pool is pp else "s",
                       name=f"po{it}")
        nc.tensor.matmul(po[:], accT_sb[:, it * 128:(it + 1) * 128], i65_sb[:],
                         start=True, stop=True)
        rz = pfin.tile([128, 1], F32, tag=f"rz{it}")
        nc.vector.reciprocal(rz[:], po[:, DQ:DQ + 1])
        o_t = pfin.tile([128, DQ], F32, tag=f"o{it}")
        nc.vector.tensor_scalar_mul(o_t[:], po[:, 0:DQ], rz[:])
        nc.gpsimd.dma_start(out[it * 128:(it + 1) * 128, :], o_t[:])


_CACHE = {}


def _program():
    if "nc" not in _CACHE:
        import contextlib
        nc = bacc.Bacc("TRN2", target_bir_lowering=False, debug=False,
                       num_devices=NCORES)
        with tile.TileContext(nc) as tc:
            with contextlib.ExitStack() as ctx:
                _emit(nc, tc, ctx)
        nc.compile()
        _CACHE["nc"] = nc
    return _CACHE["nc"]


def kernel(**inputs):
    x = np.asarray(inputs["x"], dtype=np.float32)
    ei = np.asarray(inputs["edge_index"])
    Wq = np.asarray(inputs["Wq"], dtype=np.float32)
    bq = np.asarray(inputs["bq"], dtype=np.float32)
    Wk = np.asarray(inputs["Wk"], dtype=np.float32)
    bk = np.asarray(inputs["bk"], dtype=np.float32)
    Wv = np.asarray(inputs["Wv"], dtype=np.float32)
    bv = np.asarray(inputs["bv"], dtype=np.float32)

    scale = 1.0 / np.sqrt(np.float32(DQ))
    xT16 = np.ascontiguousarray(x.T.astype(np.float16))   # (256, 8192)
    wq16 = np.ascontiguousarray((Wq * scale).astype(np.float16))
    wk16 = np.ascontiguousarray(Wk.astype(np.float16))
    wv16 = np.ascontiguousarray(Wv.astype(np.float16))
    bq_s = np.ascontiguousarray((bq * scale).reshape(DQ, 1))
    bk_c = np.ascontiguousarray(bk.reshape(DQ, 1))
    i65 = np.eye(DQ + 1, dtype=np.float32)
    i65[DQ, :DQ] = bv
    i65_16 = i65.astype(np.float16)
    adj = np.zeros((N, N), dtype=np.bool_)
    adj[ei[0], ei[1]] = True

    in_maps = []
    for c in range(NCORES):
        rows = slice(c * NLOC, (c + 1) * NLOC)
        in_maps.append({
            "xt": xT16,
            "xtq": np.ascontiguousarray(xT16[:, rows]),
            "wq": wq16, "wk": wk16, "wv": wv16,
            "bq": bq_s, "bk": bk_c, "i65": i65_16,
            "maskt": adj[rows].T.astype(np.float16),
        })

    global _last_in_maps
    _last_in_maps = in_maps
    nc = _program()
    res = run_bass_kernel_spmd(nc, in_maps, core_ids=list(range(NCORES)))
    out = np.concatenate([res.results[c]["out"] for c in range(NCORES)], axis=0)
    return out.astype(np.float32)


_last_in_maps = None
